# revision 5
# baseline (speedup 1.0000x reference)
"""Trainium2 Bass kernel for nn_ControlledNODE (sequential 16-dim neural-ODE scan).

The recurrence h_{t+1} = h_t + dt*W3 tanh(W2 tanh(W1 [h;u_t])) is strictly
sequential (measured: no contraction, Newton multiple-shooting diverges), so
one core runs the scan as a latency-optimized ping-pong between the PE and
ACT engines.  Reformulated in "q-space" (q_t = W1h h_t + W1u u_t + b1):

    ptmp_t = Maug @ z2aug_t + W1u @ du_t          (PE, fresh PSUM group)
    z1     = tanh(ptmp_{t-1} + q_{t-1})           (ACT, bias = q tile)
    psb    = W2 @ z1                              (PE)
    z2     = tanh(psb + b2)                       (ACT)
    q_t    = q_{t-1} + ptmp_{t-1}                 (DVE, off critical path)
    hist_t = [dt*W3; dt*Wro*W3] @ z2aug           (PE + DVE copy, off path)

which gives a 4-hop cross-engine chain per step.  h_t itself is never formed
on device; the host reconstructs the three readouts and h_T by prefix-summing
the per-step increments stored in `hist`.
"""

import numpy as np

T = 100000
HDIM, UDIM, HID = 16, 7, 64
DT = np.float32(5.0 / 60.0)

R = 128                      # steps per dynamic-loop iteration
NIT = (T + R - 1) // R       # 782
TPAD = NIT * R               # 100096

_CACHE = {}


def _build(nc_mod):
    import concourse.mybir as mybir
    from concourse import bacc
    from concourse.tile import TileContext

    nc = bacc.Bacc("TRN2", target_bir_lowering=False, debug=False)
    f32 = mybir.dt.float32

    du_d = nc.dram_tensor("du", (UDIM, TPAD), f32, kind="ExternalInput").ap()
    maugT_d = nc.dram_tensor("maugT", (HID + 1, HID), f32, kind="ExternalInput").ap()
    w2T_d = nc.dram_tensor("w2T", (HID, HID), f32, kind="ExternalInput").ap()
    w3aT_d = nc.dram_tensor("w3aT", (HID + 1, 19), f32, kind="ExternalInput").ap()
    w1uT_d = nc.dram_tensor("w1uT", (UDIM, HID), f32, kind="ExternalInput").ap()
    q0_d = nc.dram_tensor("q0", (HID, 1), f32, kind="ExternalInput").ap()
    b2_d = nc.dram_tensor("b2c", (HID, 1), f32, kind="ExternalInput").ap()
    hist_d = nc.dram_tensor("hist", (19, TPAD), f32, kind="ExternalOutput").ap()

    from concourse.bass import ds

    Tanh = mybir.ActivationFunctionType.Tanh

    with TileContext(nc) as tc:
        with (
            tc.tile_pool(name="const", bufs=1) as cpool,
            tc.tile_pool(name="state", bufs=1) as spool,
            tc.tile_pool(name="duc", bufs=3) as dupool,
            tc.tile_pool(name="histc", bufs=3) as hpool,
            tc.tile_pool(name="ptmp", bufs=2, space="PSUM") as ptmp_pool,
            tc.tile_pool(name="pb", bufs=2, space="PSUM") as pb_pool,
            tc.tile_pool(name="pd", bufs=2, space="PSUM") as pd_pool,
        ):
            maugT = cpool.tile([HID + 1, HID], f32, tag="maugT")
            w2T = cpool.tile([HID, HID], f32, tag="w2T")
            w3aT = cpool.tile([HID + 1, 19], f32, tag="w3aT")
            w1uT = cpool.tile([UDIM, HID], f32, tag="w1uT")
            b2c = cpool.tile([HID, 1], f32, tag="b2c")
            nc.sync.dma_start(maugT[:], maugT_d[:])
            nc.sync.dma_start(w2T[:], w2T_d[:])
            nc.sync.dma_start(w3aT[:], w3aT_d[:])
            nc.sync.dma_start(w1uT[:], w1uT_d[:])
            nc.sync.dma_start(b2c[:], b2_d[:])

            qa = spool.tile([HID, 1], f32, tag="qa")
            qb = spool.tile([HID, 1], f32, tag="qb")
            z1 = spool.tile([HID, 1], f32, tag="z1")
            z2aug = spool.tile([HID + 1, 1], f32, tag="z2aug")
            nc.sync.dma_start(qa[:], q0_d[:])
            nc.vector.memset(z2aug[0:HID, :], 0.0)
            nc.vector.memset(z2aug[HID : HID + 1, :], 1.0)

            q_tiles = [qa, qb]

            with tc.For_i(0, TPAD, R) as iv:
                du_chunk = dupool.tile([UDIM, R], f32, tag="duc")
                nc.sync.dma_start(du_chunk[:], du_d[:, ds(iv, R)])
                hist_chunk = hpool.tile([19, R], f32, tag="histc")
                for j in range(R):
                    q_old = q_tiles[j % 2]
                    q_new = q_tiles[(j + 1) % 2]
                    ptmp = ptmp_pool.tile([HID, 1], f32, tag="ptmp")
                    nc.tensor.matmul(ptmp[:], maugT[:], z2aug[:], start=True, stop=False)
                    nc.tensor.matmul(
                        ptmp[:], w1uT[:], du_chunk[:, j : j + 1], start=False, stop=True
                    )
                    # z1 = tanh(ptmp + q_old)   [main chain]
                    nc.scalar.activation(z1[:], ptmp[:], Tanh, bias=q_old[:])
                    # q_new = q_old + ptmp      [off chain, DVE]
                    nc.vector.tensor_add(q_new[:], q_old[:], ptmp[:])
                    psb = pb_pool.tile([HID, 1], f32, tag="pb")
                    nc.tensor.matmul(psb[:], w2T[:], z1[:], start=True, stop=True)
                    nc.scalar.activation(z2aug[0:HID, :], psb[:], Tanh, bias=b2c[:])
                    pd = pd_pool.tile([19, 1], f32, tag="pd")
                    nc.tensor.matmul(pd[:], w3aT[:], z2aug[:], start=True, stop=True)
                    nc.vector.tensor_copy(hist_chunk[:, j : j + 1], pd[:])
                nc.sync.dma_start(hist_d[:, ds(iv, R)], hist_chunk[:])

    nc.compile()
    return nc


def _get_nc():
    if "nc" not in _CACHE:
        _CACHE["nc"] = _build(None)
    return _CACHE["nc"]


def _prep_inputs(U, W1, b1, W2, b2, W3, b3, wd, bd, wt, bt, wc, bc, h0):
    f = np.float32
    U = np.asarray(U, f)
    W1 = np.asarray(W1, f)
    W1h = np.ascontiguousarray(W1[:, :HDIM])
    W1u = np.ascontiguousarray(W1[:, HDIM:])
    W2 = np.asarray(W2, f)
    W3 = np.asarray(W3, f)
    b1 = np.asarray(b1, f)
    b2 = np.asarray(b2, f)
    b3 = np.asarray(b3, f)
    h0 = np.asarray(h0, f)
    Wro = np.stack([np.asarray(wd, f), np.asarray(wt, f), np.asarray(wc, f)])  # (3,16)

    M = (W1h @ (DT * W3)).astype(f)                              # (64,64)
    w1hb3 = (DT * (W1h @ b3)).astype(f)                          # (64,)
    maug = np.concatenate([M, w1hb3[:, None]], axis=1)           # (64,65)
    w3a = np.concatenate([DT * W3, DT * (Wro @ W3)], axis=0)     # (19,64)
    w3b = (DT * np.concatenate([b3, Wro @ b3])).astype(f)        # (19,)
    w3aug = np.concatenate([w3a, w3b[:, None]], axis=1)          # (19,65)
    q0p = (W1h @ h0 + W1u @ U[0] + b1 - w1hb3).astype(f)         # (64,)

    du = np.zeros((UDIM, TPAD), f)
    du[:, 1:T] = (U[1:] - U[:-1]).T

    shapes = {
        "du": (UDIM, TPAD), "maugT": (HID + 1, HID), "w2T": (HID, HID),
        "w3aT": (HID + 1, 19), "w1uT": (UDIM, HID), "q0": (HID, 1),
        "b2c": (HID, 1),
    }
    inmap = {
        "du": du,
        "maugT": np.ascontiguousarray(maug.T),
        "w2T": np.ascontiguousarray(W2.T),
        "w3aT": np.ascontiguousarray(w3aug.T),
        "w1uT": np.ascontiguousarray(W1u.T),    # (7,64): lhsT for W1u@du
        "q0": q0p[:, None].copy(),
        "b2c": np.asarray(b2, f)[:, None].copy(),
    }
    for k, v in inmap.items():
        assert v.shape == shapes[k], f"{k}: {v.shape} != {shapes[k]}"
        assert v.dtype == np.float32
    return inmap, Wro, h0


def _postprocess(hist, Wro, h0, bd, bt, bc):
    f = np.float32
    hist = hist[:, :T]
    dh = hist[:16]
    dy = hist[16:]
    y0 = (Wro @ h0).astype(f)
    pref = np.concatenate(
        [np.zeros((3, 1), f), np.cumsum(dy, axis=1, dtype=f)[:, :-1]], axis=1
    )
    y = pref + y0[:, None]
    bias = np.array([bd, bt, bc], f)
    y = y + bias[:, None]
    delay, taxi, loglam = y[0].copy(), y[1].copy(), y[2].copy()
    hT = (h0 + dh.sum(axis=1, dtype=f)).astype(f).reshape(1, HDIM)
    return delay, taxi, loglam, hT


def kernel(U, W1, b1, W2, b2, W3, b3, wd, bd, wt, bt, wc, bc, h0):
    from concourse import bass_utils

    inmap, Wro, h0f = _prep_inputs(
        U, W1, b1, W2, b2, W3, b3, wd, bd, wt, bt, wc, bc, h0
    )
    nc = _get_nc()
    res = bass_utils.run_bass_kernel_spmd(
        nc, [inmap] * 8, core_ids=list(range(8))
    )
    hist = np.asarray(res.results[0]["hist"], np.float32)
    return _postprocess(hist, Wro, h0f, bd, bt, bc)
